# Initial kernel scaffold
#
"""Two-layer GAT (GATConv x2, 4 heads) for Trainium2, distributed over 8
NeuronCores.

Strategy: nodes are permuted and dealt to 8 cores (~12.5k dst nodes each).
A per-node feature-table row [h(128) | attn_src(4) | attn_dst(4) | pad] in
fp16 (512B) is produced per layer by a fused matmul (weights folded with the
attention vectors host-side), AllGathered, then each core gathers its
incoming-edge source rows with int16-indexed DMA gathers against the four
table quarters (slot layout [partition=dst, slot]), computes the edge softmax
entirely with per-partition ops (exp without max-subtraction is safe for this
value range; padding slots point at a pad row with attn_src = -60000 so
exp() zeroes them), and aggregates messages by accumulating per-slot matmuls
into PSUM. Layer outputs feed the second layer's table; final rows are
written per core and reassembled host-side.
"""

import sys

for _p in ("/opt/trn_rl_repo",):
    if _p not in sys.path:
        sys.path.insert(0, _p)

import numpy as np

import concourse.bacc as bacc
import concourse.bass as bass
import concourse.mybir as mybir
import concourse.tile as tile
from concourse.bass_utils import run_bass_kernel_spmd

F32 = mybir.dt.float32
F16 = mybir.dt.float16
I16 = mybir.dt.int16

NC = 8
NQ = 4
HEADS = 4


def build_plan(edge_index, N, F=128):
    """Host-side graph partitioning. Returns plan dict."""
    src = np.asarray(edge_index[0], dtype=np.int64)
    dst = np.asarray(edge_index[1], dtype=np.int64)
    E = src.shape[0]

    NPAD = ((N + 128 * NC - 1) // (128 * NC)) * 128 * NC
    PER = NPAD // NC          # nodes per core
    ROWS = PER + 1            # table rows per core (last = pad row)
    QROWS = 2 * ROWS          # table rows per quarter
    ntiles = PER // 128
    assert QROWS - 1 <= 32766, "in-quarter idx must fit int16"

    deg = np.bincount(dst, minlength=NPAD)
    order = np.argsort(-deg, kind="stable")
    core_of = np.empty(NPAD, dtype=np.int32)
    rank_of = np.empty(NPAD, dtype=np.int32)

    # quarter-of needs core assignment; core = deal by degree rank
    for c in range(NC):
        nodes = order[c::NC]
        core_of[nodes] = c

    qd = np.zeros((NPAD, NQ), dtype=np.int32)
    np.add.at(qd, (dst, core_of[src] // 2), 1)

    # within-core sort by quarter profile (same keys as measured padcalc)
    for c in range(NC):
        nodes = order[c::NC]
        keys = (qd[nodes, 3], qd[nodes, 2], qd[nodes, 1], qd[nodes, 0],
                qd[nodes].max(axis=1))
        srt = np.lexsort(keys)
        nodes_s = nodes[srt]
        rank_of[nodes_s] = np.arange(len(nodes_s), dtype=np.int32)

    # per-(core, tile, q) slot counts
    Dc = np.zeros((NC, ntiles, NQ), dtype=np.int32)
    for c in range(NC):
        sel = core_of[dst] == c
        ld = rank_of[dst[sel]]
        q = core_of[src[sel]] // 2
        t = ld // 128
        d = ld % 128
        cnt = np.zeros((ntiles, 128, NQ), dtype=np.int32)
        np.add.at(cnt, (t, d, q), 1)
        Dc[c] = cnt.max(axis=1)
    D = Dc.max(axis=0)  # [ntiles, NQ] shared compile-time slot counts

    PAD_I = PER  # in-quarter pad idx: even core's pad row sits at local PER

    # ---- group tiles into super-tiles with uniform per-quarter D ----
    MAX_TOT = 36        # slots per group (SBUF cap)
    MAX_STDQ = 24       # per-quarter call cap: 128*ST*Dq <= 3072
    MAX_ST = 3
    groups = []  # (t0, ST, Dg[4])
    t = 0
    while t < ntiles:
        st = 1
        Dg = D[t].copy()
        while t + st < ntiles and st < MAX_ST:
            cand = np.maximum(Dg, D[t + st])
            if (st + 1) * int(cand.sum()) > MAX_TOT:
                break
            if (st + 1) * int(cand.max()) > MAX_STDQ:
                break
            Dg = cand
            st += 1
        groups.append((t, st, Dg.copy()))
        t += st

    # group col layout: per group, per quarter, ni = 128*ST*Dq
    gcol_off = [0]
    for (t0, st, Dg) in groups:
        cols_g = sum(8 * st * int(Dg[q]) for q in range(NQ))
        gcol_off.append(gcol_off[-1] + cols_g)
    TOTCOLS = gcol_off[-1]

    idx_cores = []
    for c in range(NC):
        sel = core_of[dst] == c
        s_c = src[sel]
        d_c = dst[sel]
        ld = rank_of[d_c]
        q_c = core_of[s_c] // 2
        t_c = ld // 128
        drow = ld % 128
        inq = (core_of[s_c] % 2) * ROWS + rank_of[s_c]
        key = (t_c.astype(np.int64) * 128 + drow) * NQ + q_c
        o2 = np.argsort(key, kind="stable")
        key_s = key[o2]
        j = np.arange(len(key_s)) - np.concatenate(
            [[0], np.cumsum(np.bincount(key_s))[:-1]])[key_s]
        ixflat = np.full((128 * TOTCOLS,), PAD_I, dtype=np.int16)
        # per-tile -> group metadata arrays
        g_of_t = np.zeros(ntiles, np.int64)
        st_of_t = np.zeros(ntiles, np.int64)
        for gi, (t0, st, Dg) in enumerate(groups):
            g_of_t[t0:t0 + st] = gi
            st_of_t[t0:t0 + st] = np.arange(st)
        Dg_arr = np.zeros((len(groups), NQ), np.int64)
        ST_arr = np.zeros(len(groups), np.int64)
        qoff_in_idx = np.zeros((len(groups), NQ), np.int64)  # idx-entry offset
        colbase = np.zeros(len(groups), np.int64)
        colsg = np.zeros(len(groups), np.int64)
        for gi, (t0, st, Dg) in enumerate(groups):
            Dg_arr[gi] = Dg
            ST_arr[gi] = st
            off = 0
            for q in range(NQ):
                qoff_in_idx[gi, q] = off
                off += 128 * st * int(Dg[q])
            colbase[gi] = gcol_off[gi]
            colsg[gi] = (gcol_off[gi + 1] - gcol_off[gi])
        tt, dd, qq = t_c[o2], drow[o2], q_c[o2]
        gg = g_of_t[tt]
        ss = st_of_t[tt]
        Dq_e = Dg_arr[gg, qq]
        # position within the q-call: i = (st*Dq + j)*128 + d
        i_e = (ss * Dq_e + j) * 128 + dd
        pos = qoff_in_idx[gg, qq] + i_e     # entry within group's idx block
        grow = pos % 16
        gcol = pos // 16
        cg = colsg[gg]
        base = 128 * colbase[gg]
        for k in range(8):
            ixflat[base + (grow + 16 * k) * cg + gcol] = inq[o2].astype(np.int16)
        idx_cores.append(ixflat)

    # x slices (permuted); dummies (>=N) get zero rows
    perm_nodes = np.empty(NPAD, dtype=np.int64)
    perm_nodes[core_of * PER + rank_of] = np.arange(NPAD)
    # node at (c, r) is:
    node_at = np.empty(NPAD, dtype=np.int64)
    node_at[core_of.astype(np.int64) * PER + rank_of] = np.arange(NPAD)

    return dict(
        N=N, NPAD=NPAD, PER=PER, ROWS=ROWS, QROWS=QROWS, ntiles=ntiles,
        D=D, groups=groups, gcol_off=gcol_off, TOTCOLS=TOTCOLS,
        idx_cores=idx_cores, core_of=core_of, rank_of=rank_of,
        node_at=node_at, F=F,
    )


def build_nc(plan, check_mode=False):
    """Build the SPMD bass program."""
    PER, ROWS, QROWS = plan["PER"], plan["ROWS"], plan["QROWS"]
    ntiles, D = plan["ntiles"], plan["D"]
    groups, gcol_off = plan["groups"], plan["gcol_off"]
    TOTCOLS, F = plan["TOTCOLS"], plan["F"]
    TBL = NC * ROWS

    nc = bacc.Bacc("TRN2", target_bir_lowering=False, debug=False,
                   num_devices=NC, num_swdge_queues=4)

    x_in = nc.dram_tensor("x_slice", [PER, F], F32, kind="ExternalInput")
    idx_in = nc.dram_tensor("idx", [max(128 * TOTCOLS, 128)], I16,
                            kind="ExternalInput")
    w1_in = nc.dram_tensor("w1ext", [F, F + 8], F32, kind="ExternalInput")
    w2_in = nc.dram_tensor("w2ext", [F, F + 8], F16, kind="ExternalInput")
    b1_in = nc.dram_tensor("b1t", [128, F], F32, kind="ExternalInput")
    b1c_in = nc.dram_tensor("b1c", [128, 1], F32, kind="ExternalInput")
    b2_in = nc.dram_tensor("b2t", [128, F], F32, kind="ExternalInput")
    idf_in = nc.dram_tensor("identf", [128, 128], F32, kind="ExternalInput")
    idh_in = nc.dram_tensor("identh", [128, 128], F16, kind="ExternalInput")
    pad_in = nc.dram_tensor("padrow", [1, 256], F16, kind="ExternalInput")
    out_dram = nc.dram_tensor("out", [PER, F], F32, kind="ExternalOutput")

    EXT = F + 8  # 136

    with tile.TileContext(nc) as tc:
        with (
            tc.tile_pool(name="dram", bufs=1, space="DRAM") as dram,
            tc.tile_pool(name="consts", bufs=1) as cpool,
            tc.tile_pool(name="sbuf", bufs=2) as pool,
            tc.tile_pool(name="psum", bufs=2, space="PSUM") as psum,
        ):
            tbl1_mine = dram.tile([ROWS, 256], F16, name="tbl1_mine")
            tbl1_full = dram.tile([TBL, 256], F16, addr_space="Shared",
                                  name="tbl1_full")
            tbl2_mine = dram.tile([ROWS, 256], F16, name="tbl2_mine")
            tbl2_full = dram.tile([TBL, 256], F16, addr_space="Shared",
                                  name="tbl2_full")

            w1_s = cpool.tile([F, EXT], F32, name="w1_s")
            nc.sync.dma_start(out=w1_s[:], in_=w1_in[:])
            w2_s = cpool.tile([F, EXT], F16, name="w2_s")
            nc.sync.dma_start(out=w2_s[:], in_=w2_in[:])
            b1_s = cpool.tile([128, F], F32, name="b1_s")
            nc.sync.dma_start(out=b1_s[:], in_=b1_in[:])
            b1c_s = cpool.tile([128, 1], F32, name="b1c_s")
            nc.sync.dma_start(out=b1c_s[:], in_=b1c_in[:])
            b2_s = cpool.tile([128, F], F32, name="b2_s")
            nc.sync.dma_start(out=b2_s[:], in_=b2_in[:])
            idf_s = cpool.tile([128, 128], F32, name="idf_s")
            nc.sync.dma_start(out=idf_s[:], in_=idf_in[:])
            idh_s = cpool.tile([128, 128], F16, name="idh_s")
            nc.sync.dma_start(out=idh_s[:], in_=idh_in[:])
            prow_s = cpool.tile([1, 256], F16, name="prow_s")
            nc.sync.dma_start(out=prow_s[:], in_=pad_in[:])

            # ---------------- Phase A: table1 = [x@W1 | al_s | al_d] ------
            for t in range(ntiles):
                xt = pool.tile([128, F], F32, tag="xt", name=f"xt{t}", bufs=3)
                nc.sync.dma_start(out=xt[:], in_=x_in[t * 128:(t + 1) * 128, :])
                pT = psum.tile([128, 128], F32, tag="pT", name=f"pT{t}", bufs=1)
                nc.tensor.transpose(out=pT[:], in_=xt[:], identity=idf_s[:])
                xT = pool.tile([128, 128], F32, tag="xT", name=f"xT{t}", bufs=3)
                nc.scalar.activation(out=xT[:], in_=pT[:],
                                     func=mybir.ActivationFunctionType.Copy)
                ph = psum.tile([128, EXT], F32, tag="ph", name=f"ph{t}", bufs=1)
                nc.tensor.matmul(out=ph[:], lhsT=xT[:], rhs=w1_s[:],
                                 start=True, stop=True)
                hrow = pool.tile([128, 256], F16, tag="hrow", name=f"hrow{t}",
                                 bufs=3)
                nc.vector.tensor_copy(out=hrow[:, 0:EXT], in_=ph[:])
                nc.sync.dma_start(
                    out=tbl1_mine[t * 128:(t + 1) * 128, :], in_=hrow[:])
            nc.sync.dma_start(out=tbl1_mine[PER:PER + 1, :], in_=prow_s[:])

            cc_groups = [list(range(NC))]
            nc.gpsimd.collective_compute(
                "AllGather", mybir.AluOpType.bypass, replica_groups=cc_groups,
                ins=[tbl1_mine.opt()], outs=[tbl1_full.opt()])

            # ---------------- GAT layers ---------------------------------
            def gat_layer(tbl_mine, tbl_full, layerno):
                for gi, (t0, ST, Dg) in enumerate(groups):
                    DgS = [int(x) for x in Dg]
                    SDT = ST * sum(DgS)           # total slots in group
                    qslot = []                     # slot offset of quarter q
                    o = 0
                    for q in range(NQ):
                        qslot.append(o)
                        o += ST * DgS[q]
                    # own rows for the group's tiles
                    own = pool.tile([128, ST, 256], F16, tag="own",
                                    name=f"own{layerno}_{gi}", bufs=4)
                    for st in range(ST):
                        t = t0 + st
                        nc.sync.dma_start(
                            out=own[:, st, :],
                            in_=tbl_mine[t * 128:(t + 1) * 128, :])
                    if SDT == 0:
                        for st in range(ST):
                            fin_empty(t0 + st, layerno)
                        continue
                    cols = SDT * 8
                    idxT = pool.tile([128, cols], I16, tag="idxT",
                                     name=f"idxT{layerno}_{gi}", bufs=4)
                    ib = 128 * int(gcol_off[gi])
                    nc.sync.dma_start(
                        out=idxT[:],
                        in_=idx_in[ib:ib + 128 * cols].rearrange(
                            "(p c) -> p c", p=128))
                    G = pool.tile([128, SDT, 256], F16, tag="G",
                                  name=f"G{layerno}_{gi}", bufs=4)
                    for q in range(NQ):
                        if DgS[q] == 0:
                            continue
                        ni = 128 * ST * DgS[q]
                        nc.gpsimd.dma_gather(
                            out_ap=G[:, qslot[q]:qslot[q] + ST * DgS[q], :],
                            in_ap=tbl_full[q * QROWS:(q + 1) * QROWS, :],
                            idxs_ap=idxT[:, qslot[q] * 8:(qslot[q] + ST * DgS[q]) * 8],
                            num_idxs=ni, num_idxs_reg=ni, elem_size=256,
                            single_packet=False, queue_num=q)

                    # w holds pre-activation then exp, tile-major [ST, 4, DT]
                    DT = sum(DgS)
                    w = pool.tile([128, ST, 4, DT], F32, tag="w",
                                  name=f"w{layerno}_{gi}", bufs=3)
                    for q in range(NQ):
                        dq = DgS[q]
                        if dq == 0:
                            continue
                        g_als = G[:, qslot[q]:qslot[q] + ST * dq, F:F + 4
                                  ].rearrange("p (s j) f -> p s j f", s=ST)
                        alD = own[:, :, 132:136].rearrange(
                            "p s (o f) -> p s o f", o=1
                        ).to_broadcast([128, ST, dq, 4])
                        qo = sum(DgS[:q])
                        wout = w[:, :, :, qo:qo + dq].rearrange(
                            "p s h j -> p s j h")
                        nc.vector.tensor_tensor(
                            out=wout, in0=g_als, in1=alD,
                            op=mybir.AluOpType.add)
                    # lrelu then exp in place over the whole group
                    nc.scalar.activation(
                        out=w[:], in_=w[:],
                        func=mybir.ActivationFunctionType.Prelu, alpha=0.2)
                    nc.scalar.activation(
                        out=w[:], in_=w[:],
                        func=mybir.ActivationFunctionType.Exp)
                    den = pool.tile([128, ST, 4], F32, tag="den",
                                    name=f"den{layerno}_{gi}", bufs=3)
                    nc.vector.reduce_sum(out=den[:], in_=w[:],
                                         axis=mybir.AxisListType.X)
                    rden = pool.tile([128, ST, 4], F32, tag="rden",
                                     name=f"rden{layerno}_{gi}", bufs=3)
                    nc.vector.tensor_scalar_max(out=rden[:], in0=den[:],
                                                scalar1=1e-16)
                    nc.vector.reciprocal(out=rden[:], in_=rden[:])
                    alpha = pool.tile([128, SDT, 4], F16, tag="alpha",
                                      name=f"alpha{layerno}_{gi}", bufs=3)
                    rdb = rden[:].rearrange("p s (h o) -> p s h o", o=1
                                            ).to_broadcast([128, ST, 4, DT])
                    for q in range(NQ):
                        dq = DgS[q]
                        if dq == 0:
                            continue
                        qo = sum(DgS[:q])
                        aout = alpha[:, qslot[q]:qslot[q] + ST * dq, :
                                     ].rearrange("p (s j) h -> p s h j", s=ST)
                        nc.vector.tensor_tensor(
                            out=aout, in0=w[:, :, :, qo:qo + dq],
                            in1=rdb[:, :, :, qo:qo + dq],
                            op=mybir.AluOpType.mult)
                    Gs = pool.tile([128, SDT, F], F16, tag="Gs",
                                   name=f"Gs{layerno}_{gi}", bufs=3)
                    ab = alpha[:].rearrange("p j (h o) -> p j h o", o=1
                                            ).to_broadcast([128, SDT, 4, 32])
                    nc.vector.tensor_tensor(
                        out=Gs[:].rearrange("p j (h f) -> p j h f", h=4),
                        in0=G[:, :, 0:F].rearrange("p j (h f) -> p j h f", h=4),
                        in1=ab, op=mybir.AluOpType.mult)

                    for st in range(ST):
                        slots_t = []
                        for q in range(NQ):
                            dq = DgS[q]
                            for j in range(dq):
                                slots_t.append(qslot[q] + st * dq + j)
                        pacc = psum.tile([128, F], F32, tag="pacc",
                                         name=f"pacc{layerno}_{gi}_{st}",
                                         bufs=3)
                        for k, sl in enumerate(slots_t):
                            nc.tensor.matmul(out=pacc[:], lhsT=Gs[:, sl, :],
                                             rhs=idh_s[:], start=(k == 0),
                                             stop=(k == len(slots_t) - 1))
                        epilogue(t0 + st, layerno, pacc)

            def fin_empty(t, layerno):
                zz = pool.tile([128, F], F32, tag="zz", name=f"zz{layerno}_{t}")
                nc.vector.memset(zz[:], 0.0)
                epilogue(t, layerno, None, zeros=zz)

            def epilogue(t, layerno, pacc, zeros=None):
                # acc is transposed: [feature(part), dst(free)]
                acc_ap = pacc[:] if pacc is not None else zeros[:]
                if layerno == 1:
                    # h2^T = relu(acc^T + b1) with per-partition bias, one ACT op
                    hrT = pool.tile([128, 128], F16, tag="hrT",
                                    name=f"hrT{t}", bufs=3)
                    nc.scalar.activation(out=hrT[:], in_=acc_ap,
                                         func=mybir.ActivationFunctionType.Relu,
                                         bias=b1c_s[:, 0:1])
                    ph2 = psum.tile([128, EXT], F32, tag="ph2",
                                    name=f"ph2_{t}", bufs=2)
                    nc.tensor.matmul(out=ph2[:], lhsT=hrT[:], rhs=w2_s[:],
                                     start=True, stop=True)
                    hrow2 = pool.tile([128, 256], F16, tag="hrow2",
                                      name=f"hrow2_{t}", bufs=3)
                    nc.vector.tensor_copy(out=hrow2[:, 0:EXT], in_=ph2[:])
                    nc.sync.dma_start(
                        out=tbl2_mine[t * 128:(t + 1) * 128, :],
                        in_=hrow2[:])
                else:
                    # transpose back to [dst, feature] then add b2
                    pT2 = psum.tile([128, 128], F32, tag="pT2",
                                    name=f"pT2_{t}", bufs=1)
                    if pacc is not None:
                        accs = pool.tile([128, F], F32, tag="accs",
                                         name=f"accs{t}", bufs=3)
                        nc.scalar.activation(
                            out=accs[:], in_=acc_ap,
                            func=mybir.ActivationFunctionType.Copy)
                        nc.tensor.transpose(out=pT2[:], in_=accs[:],
                                            identity=idf_s[:])
                        src_ap = pT2[:]
                    else:
                        src_ap = acc_ap
                    ob = pool.tile([128, F], F32, tag="ob",
                                   name=f"ob{t}", bufs=3)
                    nc.vector.tensor_tensor(out=ob[:], in0=src_ap, in1=b2_s[:],
                                            op=mybir.AluOpType.add)
                    nc.sync.dma_start(
                        out=out_dram[t * 128:(t + 1) * 128, :], in_=ob[:])

            gat_layer(tbl1_mine, tbl1_full, 1)
            nc.sync.dma_start(out=tbl2_mine[PER:PER + 1, :], in_=prow_s[:])
            nc.gpsimd.collective_compute(
                "AllGather", mybir.AluOpType.bypass, replica_groups=cc_groups,
                ins=[tbl2_mine.opt()], outs=[tbl2_full.opt()])
            gat_layer(tbl2_mine, tbl2_full, 2)

    nc.compile()
    return nc


def make_weight_ext(W, a_src, a_dst):
    """[F, F'] + attention folds -> [F, F'+8] f32."""
    heads, c = a_src.shape
    Fo = W.shape[1]
    As = np.zeros((Fo, heads), dtype=np.float64)
    Ad = np.zeros((Fo, heads), dtype=np.float64)
    for h in range(heads):
        As[h * c:(h + 1) * c, h] = a_src[h]
        Ad[h * c:(h + 1) * c, h] = a_dst[h]
    return np.concatenate([W, W @ As, W @ Ad], axis=1)


def prepare_inputs(plan, x, W1, a_src1, a_dst1, b1, W2, a_src2, a_dst2, b2):
    """Build the 8 per-core input maps."""
    N, NPAD, PER, F = plan["N"], plan["NPAD"], plan["PER"], plan["F"]
    node_at = plan["node_at"]

    w1ext = make_weight_ext(np.asarray(W1, np.float64), np.asarray(a_src1),
                            np.asarray(a_dst1)).astype(np.float32)
    w2ext = make_weight_ext(np.asarray(W2, np.float64), np.asarray(a_src2),
                            np.asarray(a_dst2)).astype(np.float16)
    b1t = np.tile(np.asarray(b1, np.float32)[None, :], (128, 1))
    b2t = np.tile(np.asarray(b2, np.float32)[None, :], (128, 1))
    identf = np.eye(128, dtype=np.float32)
    identh = np.eye(128, dtype=np.float16)
    padrow = np.zeros((1, 256), dtype=np.float16)
    padrow[0, F:F + 4] = -60000.0  # al_s of pad row

    xp = np.zeros((NPAD, F), dtype=np.float32)
    valid = node_at < N
    xp[valid] = np.asarray(x, np.float32)[node_at[valid]]

    in_maps = []
    for c in range(NC):
        ixc = plan["idx_cores"][c]
        if ixc.shape[0] < 128:
            ixc = np.pad(ixc, (0, 128 - ixc.shape[0]))
        in_maps.append({
            "x_slice": xp[c * PER:(c + 1) * PER],
            "idx": ixc,
            "w1ext": w1ext, "w2ext": w2ext, "b1t": b1t, "b2t": b2t,
            "b1c": np.asarray(b1, np.float32).reshape(128, 1),
            "identf": identf, "identh": identh, "padrow": padrow,
        })
    return in_maps


def gather_output(plan, results):
    N, PER = plan["N"], plan["PER"]
    node_at = plan["node_at"]
    full = np.concatenate([results[c]["out"] for c in range(NC)], axis=0)
    out = np.zeros((N, plan["F"]), dtype=np.float32)
    valid = node_at < N
    out[node_at[valid]] = full[valid]
    return out


def run(plan, in_maps, nc=None, trace=False):
    if nc is None:
        nc = build_nc(plan)
    res = run_bass_kernel_spmd(nc, in_maps, list(range(NC)), trace=trace)
    return res


_N = 100000
_F = 128


def _run_with_retry(plan, in_maps, nc):
    try:
        return run(plan, in_maps, nc=nc)
    except Exception:
        # device may be in a bad state from a previous run; reset and retry
        try:
            import ctypes

            import jax

            jax.devices()
            lib = ctypes.CDLL("/opt/axon/libaxon_pjrt.so")
            lib.axon_reset.restype = ctypes.c_int64
            lib.axon_reset()
        except Exception:
            pass
        return run(plan, in_maps, nc=nc)


def kernel(x, edge_index, W1, a_src1, a_dst1, b1, W2, a_src2, a_dst2, b2):
    x = np.asarray(x)
    edge_index = np.asarray(edge_index)
    plan = build_plan(edge_index, x.shape[0])
    in_maps = prepare_inputs(plan, x, W1, a_src1, a_dst1, b1,
                             W2, a_src2, a_dst2, b2)
    nc = build_nc(plan)
    res = _run_with_retry(plan, in_maps, nc)
    return gather_output(plan, [res.results[c] for c in range(NC)])



# revision 17
# speedup vs baseline: 1.1482x; 1.1482x over previous
"""Two-layer GAT (GATConv x2, 4 heads) for Trainium2, distributed over 8
NeuronCores.

Strategy: nodes are permuted and dealt to 8 cores (~12.5k dst nodes each).
A per-node feature-table row of 256B holds h' = (x @ W @ diag(a_src)) in
fp16 — the attention source-logit al_s is recovered on-device as a per-head
segment sum over h', and the a_src scaling is undone for free via the
per-partition `scale` operand of the epilogue activation (the aggregated
accumulator is feature-major). Tables are AllGathered; each core gathers its
incoming-edge source rows with int16-indexed 256B DMA gathers against the
four table quarters (slot layout [partition=dst, slot]), computes the edge
softmax per-partition (exp without max-subtraction; padding slots point at a
pad row whose h' sums to -234 per head, so exp() ~ 1e-20: negligible but
positive, keeping denominators finite for zero-degree dummy rows), weights
messages in-place, and aggregates by accumulating per-slot transpose-matmuls
into PSUM. Layer outputs feed the second layer's table; the final output is
written feature-major and transposed host-side.
"""

import sys

for _p in ("/opt/trn_rl_repo",):
    if _p not in sys.path:
        sys.path.insert(0, _p)

import numpy as np

import concourse.bacc as bacc
import concourse.bass as bass
import concourse.mybir as mybir
import concourse.tile as tile
from concourse.bass_utils import run_bass_kernel_spmd

F32 = mybir.dt.float32
F16 = mybir.dt.float16
I16 = mybir.dt.int16

NC = 8
NQ = 4
HEADS = 4
PADVAL = -7.3125  # pad-row h' element: per-head sum = 32*PADVAL = -234


def build_plan(edge_index, N, F=128):
    """Host-side graph partitioning. Returns plan dict."""
    src = np.asarray(edge_index[0], dtype=np.int64)
    dst = np.asarray(edge_index[1], dtype=np.int64)

    NPAD = ((N + 128 * NC - 1) // (128 * NC)) * 128 * NC
    PER = NPAD // NC          # nodes per core
    ROWS = PER + 1            # table rows per core (last = pad row)
    QROWS = 2 * ROWS          # table rows per quarter
    ntiles = PER // 128
    assert QROWS - 1 <= 32766, "in-quarter idx must fit int16"

    deg = np.bincount(dst, minlength=NPAD)
    order = np.argsort(-deg, kind="stable")
    core_of = np.empty(NPAD, dtype=np.int32)
    rank_of = np.empty(NPAD, dtype=np.int32)

    # core = deal by degree rank
    for c in range(NC):
        nodes = order[c::NC]
        core_of[nodes] = c

    qd = np.zeros((NPAD, NQ), dtype=np.int32)
    np.add.at(qd, (dst, core_of[src] // 2), 1)

    # within-core sort by quarter profile
    for c in range(NC):
        nodes = order[c::NC]
        keys = (qd[nodes, 3], qd[nodes, 2], qd[nodes, 1], qd[nodes, 0],
                qd[nodes].max(axis=1))
        srt = np.lexsort(keys)
        nodes_s = nodes[srt]
        rank_of[nodes_s] = np.arange(len(nodes_s), dtype=np.int32)

    # per-(core, tile, q) slot counts
    Dc = np.zeros((NC, ntiles, NQ), dtype=np.int32)
    for c in range(NC):
        sel = core_of[dst] == c
        ld = rank_of[dst[sel]]
        q = core_of[src[sel]] // 2
        t = ld // 128
        d = ld % 128
        cnt = np.zeros((ntiles, 128, NQ), dtype=np.int32)
        np.add.at(cnt, (t, d, q), 1)
        Dc[c] = cnt.max(axis=1)
    D = Dc.max(axis=0)  # [ntiles, NQ] shared compile-time slot counts

    PAD_I = PER  # in-quarter pad idx: even core's pad row sits at local PER

    # ---- group tiles into chunks with uniform per-quarter D ----
    MAX_TOT = 80        # slots per group (SBUF cap)
    MAX_STDQ = 24       # per-quarter call cap: 128*ST*Dq <= 3072
    MAX_ST = 4          # bounded by PSUM rotation (pacc/ph2 live 2 chunks)
    groups = []  # (t0, ST, Dg[4])
    t = 0
    while t < ntiles:
        st = 1
        Dg = D[t].copy()
        while t + st < ntiles and st < MAX_ST:
            cand = np.maximum(Dg, D[t + st])
            if (st + 1) * int(cand.sum()) > MAX_TOT:
                break
            if (st + 1) * int(cand.max()) > MAX_STDQ:
                break
            Dg = cand
            st += 1
        groups.append((t, st, Dg.copy()))
        t += st

    # group col layout: per group, per quarter, ni = 128*ST*Dq
    gcol_off = [0]
    for (t0, st, Dg) in groups:
        cols_g = sum(8 * st * int(Dg[q]) for q in range(NQ))
        gcol_off.append(gcol_off[-1] + cols_g)
    TOTCOLS = gcol_off[-1]

    idx_cores = []
    for c in range(NC):
        sel = core_of[dst] == c
        s_c = src[sel]
        d_c = dst[sel]
        ld = rank_of[d_c]
        q_c = core_of[s_c] // 2
        t_c = ld // 128
        drow = ld % 128
        inq = (core_of[s_c] % 2) * ROWS + rank_of[s_c]
        key = (t_c.astype(np.int64) * 128 + drow) * NQ + q_c
        o2 = np.argsort(key, kind="stable")
        key_s = key[o2]
        j = np.arange(len(key_s)) - np.concatenate(
            [[0], np.cumsum(np.bincount(key_s))[:-1]])[key_s]
        ixflat = np.full((128 * TOTCOLS,), PAD_I, dtype=np.int16)
        # per-tile -> group metadata arrays
        g_of_t = np.zeros(ntiles, np.int64)
        st_of_t = np.zeros(ntiles, np.int64)
        for gi, (t0, st, Dg) in enumerate(groups):
            g_of_t[t0:t0 + st] = gi
            st_of_t[t0:t0 + st] = np.arange(st)
        Dg_arr = np.zeros((len(groups), NQ), np.int64)
        qoff_in_idx = np.zeros((len(groups), NQ), np.int64)
        colbase = np.zeros(len(groups), np.int64)
        colsg = np.zeros(len(groups), np.int64)
        for gi, (t0, st, Dg) in enumerate(groups):
            Dg_arr[gi] = Dg
            off = 0
            for q in range(NQ):
                qoff_in_idx[gi, q] = off
                off += 128 * st * int(Dg[q])
            colbase[gi] = gcol_off[gi]
            colsg[gi] = (gcol_off[gi + 1] - gcol_off[gi])
        tt, dd, qq = t_c[o2], drow[o2], q_c[o2]
        gg = g_of_t[tt]
        ss = st_of_t[tt]
        Dq_e = Dg_arr[gg, qq]
        # position within the q-call: i = (st*Dq + j)*128 + d
        i_e = (ss * Dq_e + j) * 128 + dd
        pos = qoff_in_idx[gg, qq] + i_e     # entry within group's idx block
        grow = pos % 16
        gcol = pos // 16
        cg = colsg[gg]
        base = 128 * colbase[gg]
        for k in range(8):
            ixflat[base + (grow + 16 * k) * cg + gcol] = inq[o2].astype(np.int16)
        idx_cores.append(ixflat)

    node_at = np.empty(NPAD, dtype=np.int64)
    node_at[core_of.astype(np.int64) * PER + rank_of] = np.arange(NPAD)

    return dict(
        N=N, NPAD=NPAD, PER=PER, ROWS=ROWS, QROWS=QROWS, ntiles=ntiles,
        D=D, groups=groups, gcol_off=gcol_off, TOTCOLS=TOTCOLS,
        idx_cores=idx_cores, core_of=core_of, rank_of=rank_of,
        node_at=node_at, F=F,
    )


def build_nc(plan, check_mode=False):
    """Build the SPMD bass program."""
    PER, ROWS, QROWS = plan["PER"], plan["ROWS"], plan["QROWS"]
    ntiles = plan["ntiles"]
    groups, gcol_off = plan["groups"], plan["gcol_off"]
    TOTCOLS, F = plan["TOTCOLS"], plan["F"]
    TBL = NC * ROWS
    EXT = F + 4  # 132: h' columns + al_d columns

    nc = bacc.Bacc("TRN2", target_bir_lowering=False, debug=False,
                   num_devices=NC, num_swdge_queues=4)

    xT_in = nc.dram_tensor("xT_slice", [F, PER], F32, kind="ExternalInput")
    idx_in = nc.dram_tensor("idx", [max(128 * TOTCOLS, 128)], I16,
                            kind="ExternalInput")
    w1_in = nc.dram_tensor("w1ext", [F, EXT], F32, kind="ExternalInput")
    w2_in = nc.dram_tensor("w2ext", [F, EXT], F16, kind="ExternalInput")
    b1c_in = nc.dram_tensor("b1c", [128, 1], F32, kind="ExternalInput")
    b2c_in = nc.dram_tensor("b2c", [128, 1], F32, kind="ExternalInput")
    inv1_in = nc.dram_tensor("inv1", [128, 1], F32, kind="ExternalInput")
    inv2_in = nc.dram_tensor("inv2", [128, 1], F32, kind="ExternalInput")
    idh_in = nc.dram_tensor("identh", [128, 128], F16, kind="ExternalInput")
    pad1_in = nc.dram_tensor("padrow1", [1, F], F16, kind="ExternalInput")
    pad2_in = nc.dram_tensor("padrow2", [1, F], F16, kind="ExternalInput")
    out_dram = nc.dram_tensor("outT", [F, PER], F32, kind="ExternalOutput")

    XCH = 8  # tiles per x-load / phase-A table-write batch

    with tile.TileContext(nc) as tc:
        with (
            tc.tile_pool(name="dram", bufs=1, space="DRAM") as dram,
            tc.tile_pool(name="consts", bufs=1) as cpool,
            tc.tile_pool(name="sbuf", bufs=2) as pool,
            tc.tile_pool(name="psum", bufs=2, space="PSUM") as psum,
        ):
            tbl1_mine = dram.tile([ROWS, F], F16, name="tbl1_mine")
            tbl1_full = dram.tile([TBL, F], F16, addr_space="Shared",
                                  name="tbl1_full")
            tbl2_mine = dram.tile([ROWS, F], F16, name="tbl2_mine")
            tbl2_full = dram.tile([TBL, F], F16, addr_space="Shared",
                                  name="tbl2_full")

            w1_s = cpool.tile([F, EXT], F32, name="w1_s")
            nc.sync.dma_start(out=w1_s[:], in_=w1_in[:])
            w2_s = cpool.tile([F, EXT], F16, name="w2_s")
            nc.sync.dma_start(out=w2_s[:], in_=w2_in[:])
            b1c_s = cpool.tile([128, 1], F32, name="b1c_s")
            nc.sync.dma_start(out=b1c_s[:], in_=b1c_in[:])
            b2c_s = cpool.tile([128, 1], F32, name="b2c_s")
            nc.sync.dma_start(out=b2c_s[:], in_=b2c_in[:])
            inv1_s = cpool.tile([128, 1], F32, name="inv1_s")
            nc.sync.dma_start(out=inv1_s[:], in_=inv1_in[:])
            inv2_s = cpool.tile([128, 1], F32, name="inv2_s")
            nc.sync.dma_start(out=inv2_s[:], in_=inv2_in[:])
            idh_s = cpool.tile([128, 128], F16, name="idh_s")
            nc.sync.dma_start(out=idh_s[:], in_=idh_in[:])
            prow1_s = cpool.tile([1, F], F16, name="prow1_s")
            nc.sync.dma_start(out=prow1_s[:], in_=pad1_in[:])
            prow2_s = cpool.tile([1, F], F16, name="prow2_s")
            nc.sync.dma_start(out=prow2_s[:], in_=pad2_in[:])

            # SBUF-resident per-(tile,head) dst attention logits
            alsd1_s = cpool.tile([128, ntiles, HEADS], F32, name="alsd1_s")
            alsd2_s = cpool.tile([128, ntiles, HEADS], F32, name="alsd2_s")

            # ---------------- Phase A: tbl1 = x @ W1ext ------------------
            nxch = (ntiles + XCH - 1) // XCH
            for b in range(nxch):
                t0 = b * XCH
                bt = min(XCH, ntiles - t0)
                xt = pool.tile([128, XCH * 128], F32, tag="xt",
                               name=f"xt{b}", bufs=2)
                nc.sync.dma_start(
                    out=xt[:, 0:bt * 128],
                    in_=xT_in[:, t0 * 128:(t0 + bt) * 128])
                harena = pool.tile([128, XCH, F], F16, tag="harena",
                                   name=f"harena{b}", bufs=2)
                for k in range(bt):
                    t = t0 + k
                    ph = psum.tile([128, EXT], F32, tag="ph",
                                   name=f"ph{t}", bufs=2)
                    nc.tensor.matmul(out=ph[:],
                                     lhsT=xt[:, k * 128:(k + 1) * 128],
                                     rhs=w1_s[:], start=True, stop=True)
                    nc.scalar.activation(out=harena[:, k, :], in_=ph[:, 0:F],
                                         func=mybir.ActivationFunctionType.Copy)
                    nc.vector.tensor_copy(out=alsd1_s[:, t, :],
                                          in_=ph[:, F:EXT])
                nc.sync.dma_start(
                    out=tbl1_mine[t0 * 128:(t0 + bt) * 128, :].rearrange(
                        "(g p) f -> p g f", p=128),
                    in_=harena[:, 0:bt, :])
            nc.sync.dma_start(out=tbl1_mine[PER:PER + 1, :], in_=prow1_s[:])

            cc_groups = [list(range(NC))]
            nc.gpsimd.collective_compute(
                "AllGather", mybir.AluOpType.bypass, replica_groups=cc_groups,
                ins=[tbl1_mine.opt()], outs=[tbl1_full.opt()])

            # ---------------- GAT layers ---------------------------------
            # Software-pipelined emission: each chunk's epilogue is emitted
            # one chunk LATE, before the next chunk's gathers, with all
            # PSUM->SBUF epilogue copies on the Scalar engine. This keeps
            # every engine's in-order stream free of ops that would block
            # on the previous chunk's aggregation matmuls.
            def gat_layer(tbl_full, layerno):
                alsd_s = alsd1_s if layerno == 1 else alsd2_s
                pend = []  # deferred epilogue state: (t0, ST, paccs)

                def flush_epilogue():
                    if not pend:
                        return
                    t0, ST, pacc_g = pend.pop()
                    if layerno == 1:
                        harena2 = pool.tile([128, ST, F], F16, tag="harena2",
                                            name=f"ha2_{layerno}_{t0}", bufs=2)
                        for st in range(ST):
                            t = t0 + st
                            hrT = pool.tile([128, 128], F16, tag="hrT",
                                            name=f"hrT{t}", bufs=3)
                            nc.scalar.activation(
                                out=hrT[:], in_=pacc_g[:, st, :],
                                func=mybir.ActivationFunctionType.Relu,
                                bias=b1c_s[:, 0:1], scale=inv1_s[:, 0:1])
                            ph2 = psum.tile([128, EXT], F32, tag="ph2",
                                            name=f"ph2_{t}", bufs=3)
                            nc.tensor.matmul(out=ph2[:], lhsT=hrT[:],
                                             rhs=w2_s[:], start=True,
                                             stop=True)
                            nc.scalar.activation(
                                out=harena2[:, st, :], in_=ph2[:, 0:F],
                                func=mybir.ActivationFunctionType.Copy)
                            nc.scalar.activation(
                                out=alsd2_s[:, t, :], in_=ph2[:, F:EXT],
                                func=mybir.ActivationFunctionType.Copy)
                        nc.sync.dma_start(
                            out=tbl2_mine[t0 * 128:(t0 + ST) * 128, :
                                          ].rearrange("(g p) f -> p g f",
                                                      p=128),
                            in_=harena2[:])
                    else:
                        oarena = pool.tile([128, ST, F], F32, tag="oarena",
                                           name=f"oa_{t0}", bufs=2)
                        for st in range(ST):
                            nc.scalar.activation(
                                out=oarena[:, st, :], in_=pacc_g[:, st, :],
                                func=mybir.ActivationFunctionType.Identity,
                                bias=b2c_s[:, 0:1], scale=inv2_s[:, 0:1])
                        nc.sync.dma_start(
                            out=out_dram[:, t0 * 128:(t0 + ST) * 128],
                            in_=oarena[:].rearrange("p s f -> p (s f)"))

                for gi, (t0, ST, Dg) in enumerate(groups):
                    DgS = [int(x) for x in Dg]
                    SDT = ST * sum(DgS)           # total slots in group
                    DT = sum(DgS)                 # slots per tile
                    qslot = []                    # slot offset of quarter q
                    o = 0
                    for q in range(NQ):
                        qslot.append(o)
                        o += ST * DgS[q]
                    if SDT == 0:
                        flush_epilogue()
                        for st in range(ST):
                            fin_empty(t0 + st, layerno)
                        continue

                    # ---- deferred epilogue of the previous chunk ----
                    flush_epilogue()

                    # ---- gathers + per-quarter attention (pipelined) ----
                    cols = SDT * 8
                    idxT = pool.tile([128, cols], I16, tag="idxT",
                                     name=f"idxT{layerno}_{gi}", bufs=5)
                    ib = 128 * int(gcol_off[gi])
                    nc.sync.dma_start(
                        out=idxT[:],
                        in_=idx_in[ib:ib + 128 * cols].rearrange(
                            "(p c) -> p c", p=128))
                    Gq = [None] * NQ
                    alsq = [None] * NQ
                    for q in range(NQ):
                        if DgS[q] == 0:
                            continue
                        nq = ST * DgS[q]
                        ni = 128 * nq
                        G = pool.tile([128, nq, F], F16, tag=f"G{q}",
                                      name=f"G{layerno}_{gi}_{q}", bufs=5)
                        Gq[q] = G
                        nc.gpsimd.dma_gather(
                            out_ap=G[:],
                            in_ap=tbl_full[q * QROWS:(q + 1) * QROWS, :],
                            idxs_ap=idxT[:, qslot[q] * 8:(qslot[q] + nq) * 8],
                            num_idxs=ni, num_idxs_reg=ni, elem_size=F,
                            single_packet=False, queue_num=q)
                        # per-edge al_s: per-head sum over h' (f16 tree add
                        # in 2x DVE mode, then reduce over 16)
                        g4 = G[:].rearrange("p n (h f) -> p n h f", h=HEADS)
                        a16 = pool.tile([128, nq, HEADS, F // HEADS // 2],
                                        F16, tag=f"a16_{q}",
                                        name=f"a16_{layerno}_{gi}_{q}", bufs=2)
                        nc.vector.tensor_tensor(
                            out=a16[:], in0=g4[:, :, :, 0:16],
                            in1=g4[:, :, :, 16:32], op=mybir.AluOpType.add)
                        als = pool.tile([128, nq, HEADS], F32, tag=f"als_{q}",
                                        name=f"als{layerno}_{gi}_{q}", bufs=3)
                        nc.vector.reduce_sum(out=als[:], in_=a16[:],
                                             axis=mybir.AxisListType.X)
                        alsq[q] = als

                    # w: logits, layout [ST, H, DT] (DT = quarter-major slots)
                    w = pool.tile([128, ST, HEADS, DT], F32, tag="w",
                                  name=f"w{layerno}_{gi}", bufs=3)
                    for q in range(NQ):
                        dq = DgS[q]
                        if dq == 0:
                            continue
                        g_als = alsq[q][:].rearrange("p (s j) h -> p s h j",
                                                     s=ST)
                        alD = alsd_s[:, t0:t0 + ST, :].rearrange(
                            "p s (o h) -> p s h o", o=1
                        ).to_broadcast([128, ST, HEADS, dq])
                        qo = sum(DgS[:q])
                        nc.vector.tensor_tensor(
                            out=w[:, :, :, qo:qo + dq], in0=g_als, in1=alD,
                            op=mybir.AluOpType.add)
                    # lrelu then exp in place over the whole group
                    nc.scalar.activation(
                        out=w[:], in_=w[:],
                        func=mybir.ActivationFunctionType.Prelu, alpha=0.2)
                    nc.scalar.activation(
                        out=w[:], in_=w[:],
                        func=mybir.ActivationFunctionType.Exp)
                    den = pool.tile([128, ST, HEADS], F32, tag="den",
                                    name=f"den{layerno}_{gi}", bufs=3)
                    nc.vector.reduce_sum(out=den[:], in_=w[:],
                                         axis=mybir.AxisListType.X)
                    rden = pool.tile([128, ST, HEADS], F32, tag="rden",
                                     name=f"rden{layerno}_{gi}", bufs=3)
                    nc.vector.reciprocal_approx_fast(out=rden[:], in_=den[:])
                    for q in range(NQ):
                        dq = DgS[q]
                        if dq == 0:
                            continue
                        nq = ST * dq
                        qo = sum(DgS[:q])
                        alpha = pool.tile([128, nq, HEADS], F16,
                                          tag=f"alpha_{q}",
                                          name=f"alpha{layerno}_{gi}_{q}",
                                          bufs=3)
                        aout = alpha[:].rearrange("p (s j) h -> p s j h",
                                                  s=ST)
                        rdb = rden[:].rearrange("p s (o h) -> p s o h", o=1
                                                ).to_broadcast(
                                                    [128, ST, dq, HEADS])
                        nc.vector.tensor_tensor(
                            out=aout,
                            in0=w[:, :, :, qo:qo + dq].rearrange(
                                "p s h j -> p s j h"),
                            in1=rdb, op=mybir.AluOpType.mult)
                        # weight messages in place: Gq *= alpha
                        gv = Gq[q][:].rearrange("p n (h f) -> p n h f",
                                                h=HEADS)
                        ab = alpha[:].rearrange("p n (h o) -> p n h o", o=1
                                                ).to_broadcast(
                                                    [128, nq, HEADS,
                                                     F // HEADS])
                        nc.vector.tensor_tensor(out=gv, in0=gv, in1=ab,
                                                op=mybir.AluOpType.mult)

                    # ---- aggregation matmuls (epilogue deferred) ----
                    pacc_g = psum.tile([128, ST, F], F32, tag="pacc",
                                       name=f"pacc{layerno}_{gi}", bufs=2)
                    for st in range(ST):
                        slots_t = []
                        for q in range(NQ):
                            dq = DgS[q]
                            for jj in range(dq):
                                slots_t.append((q, st * dq + jj))
                        for k, (q, sl) in enumerate(slots_t):
                            nc.tensor.matmul(out=pacc_g[:, st, :],
                                             lhsT=Gq[q][:, sl, :],
                                             rhs=idh_s[:], start=(k == 0),
                                             stop=(k == len(slots_t) - 1))
                    pend.append((t0, ST, pacc_g))
                flush_epilogue()

            def fin_empty(t, layerno):
                # no incoming edges anywhere in this tile: out = bias
                if layerno == 1:
                    zz = pool.tile([128, F], F16, tag="zz1", name=f"zz1_{t}")
                    nc.vector.memset(zz[:], 0.0)
                    # relu(b1) per feature-partition, broadcast over dsts
                    hb = pool.tile([128, F], F16, tag="hb", name=f"hb{t}")
                    nc.scalar.activation(
                        out=hb[:], in_=zz[:],
                        func=mybir.ActivationFunctionType.Relu,
                        bias=b1c_s[:, 0:1])
                    ph2 = psum.tile([128, EXT], F32, tag="ph2",
                                    name=f"ph2e_{t}", bufs=2)
                    nc.tensor.matmul(out=ph2[:], lhsT=hb[:], rhs=w2_s[:],
                                     start=True, stop=True)
                    hrow2 = pool.tile([128, F], F16, tag="hrow2e",
                                      name=f"hrow2e_{t}", bufs=2)
                    nc.scalar.activation(
                        out=hrow2[:], in_=ph2[:, 0:F],
                        func=mybir.ActivationFunctionType.Copy)
                    nc.vector.tensor_copy(out=alsd2_s[:, t, :],
                                          in_=ph2[:, F:EXT])
                    nc.sync.dma_start(
                        out=tbl2_mine[t * 128:(t + 1) * 128, :],
                        in_=hrow2[:])
                else:
                    # outT[f, d] = b2[f] for every dst in the tile
                    zo = pool.tile([128, F], F32, tag="zo", name=f"zo{t}")
                    nc.vector.memset(zo[:], 0.0)
                    ob = pool.tile([128, F], F32, tag="obe", name=f"obe{t}")
                    nc.scalar.activation(
                        out=ob[:], in_=zo[:],
                        func=mybir.ActivationFunctionType.Identity,
                        bias=b2c_s[:, 0:1])
                    nc.sync.dma_start(
                        out=out_dram[:, t * 128:(t + 1) * 128], in_=ob[:])

            gat_layer(tbl1_full, 1)
            nc.sync.dma_start(out=tbl2_mine[PER:PER + 1, :], in_=prow2_s[:])
            nc.gpsimd.collective_compute(
                "AllGather", mybir.AluOpType.bypass, replica_groups=cc_groups,
                ins=[tbl2_mine.opt()], outs=[tbl2_full.opt()])
            gat_layer(tbl2_full, 2)

    nc.compile()
    return nc


def make_padrow(a_flat):
    """Pad-row h' values: per head, -C on features where a_src > 0 (C chosen
    so the per-head sum is -234), 0 elsewhere. Guarantees (a) per-edge al_s
    of a pad slot is -234 (exp -> ~1e-20 after lrelu), and (b) 1/a_src *
    padval <= 0 per feature, so all-pad (dummy) rows ReLU to exactly 0 in
    the next layer instead of amplifying by 1/a_src."""
    F = a_flat.shape[0]
    C = F // HEADS
    row = np.zeros(F, dtype=np.float64)
    for h in range(HEADS):
        s = a_flat[h * C:(h + 1) * C] > 0
        npos = int(s.sum())
        assert npos > 0, "all-negative a_src head; pad-row trick inapplicable"
        row[h * C:(h + 1) * C][s] = -234.0 / npos
    return row.astype(np.float16).reshape(1, F)


def make_weight_ext(W, a_src, a_dst):
    """[F, Fo] -> [F, Fo+4]: W@diag(a_src_flat) | W@Ad."""
    heads, c = a_src.shape
    Fo = W.shape[1]
    as_flat = np.zeros(Fo, dtype=np.float64)
    Ad = np.zeros((Fo, heads), dtype=np.float64)
    for h in range(heads):
        as_flat[h * c:(h + 1) * c] = a_src[h]
        Ad[h * c:(h + 1) * c, h] = a_dst[h]
    return np.concatenate([W * as_flat[None, :], W @ Ad], axis=1)


def prepare_inputs(plan, x, W1, a_src1, a_dst1, b1, W2, a_src2, a_dst2, b2):
    """Build the 8 per-core input maps."""
    N, NPAD, PER, F = plan["N"], plan["NPAD"], plan["PER"], plan["F"]
    node_at = plan["node_at"]

    a1f = np.zeros(F, dtype=np.float64)
    a2f = np.zeros(F, dtype=np.float64)
    hs1 = np.asarray(a_src1, np.float64)
    hs2 = np.asarray(a_src2, np.float64)
    for h in range(HEADS):
        a1f[h * (F // HEADS):(h + 1) * (F // HEADS)] = hs1[h]
        a2f[h * (F // HEADS):(h + 1) * (F // HEADS)] = hs2[h]

    w1ext = make_weight_ext(np.asarray(W1, np.float64), np.asarray(a_src1),
                            np.asarray(a_dst1)).astype(np.float32)
    w2ext = make_weight_ext(np.asarray(W2, np.float64), np.asarray(a_src2),
                            np.asarray(a_dst2)).astype(np.float16)
    inv1 = (1.0 / a1f).astype(np.float32).reshape(128, 1)
    inv2 = (1.0 / a2f).astype(np.float32).reshape(128, 1)
    b1c = np.asarray(b1, np.float32).reshape(128, 1)
    b2c = np.asarray(b2, np.float32).reshape(128, 1)
    identh = np.eye(128, dtype=np.float16)
    padrow1 = make_padrow(a1f)
    padrow2 = make_padrow(a2f)

    xp = np.zeros((NPAD, F), dtype=np.float32)
    valid = node_at < N
    xp[valid] = np.asarray(x, np.float32)[node_at[valid]]

    in_maps = []
    for c in range(NC):
        ixc = plan["idx_cores"][c]
        if ixc.shape[0] < 128:
            ixc = np.pad(ixc, (0, 128 - ixc.shape[0]))
        xTc = np.ascontiguousarray(xp[c * PER:(c + 1) * PER].T)
        in_maps.append({
            "xT_slice": xTc,
            "idx": ixc,
            "w1ext": w1ext, "w2ext": w2ext,
            "b1c": b1c, "b2c": b2c, "inv1": inv1, "inv2": inv2,
            "identh": identh, "padrow1": padrow1, "padrow2": padrow2,
        })
    return in_maps


def gather_output(plan, results):
    N, PER = plan["N"], plan["PER"]
    node_at = plan["node_at"]
    full = np.concatenate(
        [np.ascontiguousarray(results[c]["outT"].T) for c in range(NC)],
        axis=0)
    out = np.zeros((N, plan["F"]), dtype=np.float32)
    valid = node_at < N
    out[node_at[valid]] = full[valid]
    return out


def run(plan, in_maps, nc=None, trace=False):
    if nc is None:
        nc = build_nc(plan)
    res = run_bass_kernel_spmd(nc, in_maps, list(range(NC)), trace=trace)
    return res


_N = 100000
_F = 128


def _run_with_retry(plan, in_maps, nc):
    try:
        return run(plan, in_maps, nc=nc)
    except Exception:
        # device may be in a bad state from a previous run; reset and retry
        try:
            import ctypes

            import jax

            jax.devices()
            lib = ctypes.CDLL("/opt/axon/libaxon_pjrt.so")
            lib.axon_reset.restype = ctypes.c_int64
            lib.axon_reset()
        except Exception:
            pass
        return run(plan, in_maps, nc=nc)


def kernel(x, edge_index, W1, a_src1, a_dst1, b1, W2, a_src2, a_dst2, b2):
    x = np.asarray(x)
    edge_index = np.asarray(edge_index)
    plan = build_plan(edge_index, x.shape[0])
    in_maps = prepare_inputs(plan, x, W1, a_src1, a_dst1, b1,
                             W2, a_src2, a_dst2, b2)
    nc = build_nc(plan)
    res = _run_with_retry(plan, in_maps, nc)
    return gather_output(plan, [res.results[c] for c in range(NC)])
